# revision 9
# baseline (speedup 1.0000x reference)
"""Masked attention kernel for Trainium2, data-parallel over batch on 8 NeuronCores.

Problem (per reference):
    query (128, 512) f32, key/value (1024, 128, 512) f32, mask (128, 1, 1024) i32
    energy = einsum('bh,tbh->bt'); attn = softmax(energy)
    masked = mask*attn / sum(mask*attn); context = einsum('bt,tbh->bh')
    returns (context (128,512), masked_attention (128,1024))

Key algebraic simplification: the unmasked softmax normalizer cancels:
    masked = m*exp(e-max) / sum(m*exp(e-max))

Per-core layout (B_loc=16, T=1024, H=512):
    - K/V streamed per-batch as (128 t-part, 8 tt, 512 h) tiles (2 MB DMAs)
    - energy via fused DVE tensor_tensor_reduce (K-tile * q_broadcast, sum over h)
      -> E_all (128 t-part, tt*16+b columns)
    - PE transposes E to row layout (16 b-part, 1024 t); softmax row-wise:
      reduce_max(negate) -> ACT Exp(bias=-max) -> ttr(*mask, sum) -> reciprocal
    - attn rows scaled by 1/Z, DMA'd out
    - W transposed back to column layout; context via PE matmuls in float32r
      (lhsT = W columns, rhs = V tiles), scaled by 1/Z at the end
"""

import numpy as np

B, T, H = 128, 1024, 512
NCORES = 8
BL = B // NCORES  # 16 batches per core
NT = T // 128     # 8 t-tiles

_cache = {}


def _build_nc(debug=False):
    from contextlib import ExitStack

    import concourse.bacc as bacc
    import concourse.bass as bass
    import concourse.mybir as mybir
    import concourse.tile as tile
    from concourse import masks

    f32 = mybir.dt.float32
    f32r = mybir.dt.float32r
    i32 = mybir.dt.int32
    Alu = mybir.AluOpType
    Act = mybir.ActivationFunctionType

    nc = bacc.Bacc("TRN2", target_bir_lowering=False, debug=debug)

    q_d = nc.dram_tensor("query", [BL, H], f32, kind="ExternalInput")
    k_d = nc.dram_tensor("key", [T, BL, H], f32, kind="ExternalInput")
    v_d = nc.dram_tensor("value", [T, BL, H], f32r, kind="ExternalInput")
    m_d = nc.dram_tensor("mask", [BL, 1, T], i32, kind="ExternalInput")
    ctx_d = nc.dram_tensor("out_ctx", [BL, H], f32, kind="ExternalOutput")
    attn_d = nc.dram_tensor("out_attn", [BL, T], f32, kind="ExternalOutput")

    # (T, BL, H) -> (BL, 128, NT, H): per-batch tiles, t on partitions
    k_r = k_d.ap().rearrange("(tt p) b h -> b p tt h", p=128)
    v_r = v_d.ap().rearrange("(tt p) b h -> b p tt h", p=128)

    with tile.TileContext(nc) as tc, ExitStack() as ctx:
        const = ctx.enter_context(tc.tile_pool(name="const", bufs=1))
        kpool = ctx.enter_context(tc.tile_pool(name="kpool", bufs=3))
        vpool = ctx.enter_context(tc.tile_pool(name="vpool", bufs=3))
        prodp = ctx.enter_context(tc.tile_pool(name="prodp", bufs=2))
        psum_e = ctx.enter_context(
            tc.tile_pool(name="psum_e", bufs=1, space=bass.MemorySpace.PSUM)
        )
        psum_w = ctx.enter_context(
            tc.tile_pool(name="psum_w", bufs=1, space=bass.MemorySpace.PSUM)
        )
        psum_c = ctx.enter_context(
            tc.tile_pool(name="psum_c", bufs=4, space=bass.MemorySpace.PSUM)
        )

        identity = const.tile([128, 128], f32)
        masks.make_identity(nc, identity[:])

        mask_i = const.tile([BL, T], i32)
        nc.sync.dma_start(mask_i[:], m_d.ap().rearrange("b o t -> b (o t)"))
        mask_f = const.tile([BL, T], f32)
        nc.vector.tensor_copy(mask_f[:], mask_i[:])

        # broadcast each query row to all 128 partitions (DMA with 0-stride read)
        qb = const.tile([128, BL, H], f32)
        for b in range(BL):
            nc.sync.dma_start(qb[:, b, :], q_d.ap()[b : b + 1, :].to_broadcast((128, H)))

        # ---- energy: E_all[p, tt*BL+b] = sum_h K[tt*128+p, b, h] * q[b, h]
        E_all = const.tile([128, NT * BL], f32)
        for b in range(BL):
            kt = kpool.tile([128, NT, H], f32, tag="kt")
            nc.sync.dma_start(kt[:], k_r[b])
            for tt in range(NT):
                # fused tensor_tensor_reduce faults on this HW path; split:
                # DVE multiply, then ACT reduce via activation accum_out
                pr = prodp.tile([128, H], f32, tag="pr")
                nc.vector.tensor_tensor(
                    out=pr[:], in0=kt[:, tt, :], in1=qb[:, b, :], op=Alu.mult
                )
                pr2 = prodp.tile([128, H], f32, tag="pr2")
                nc.scalar.activation(
                    pr2[:],
                    pr[:],
                    Act.Identity,
                    accum_out=E_all[:, tt * BL + b : tt * BL + b + 1],
                )

        # ---- transpose energy to row layout: erow[b, t]
        erow = psum_e.tile([BL, T], f32)
        for tt in range(NT):
            nc.tensor.transpose(
                erow[:, tt * 128 : (tt + 1) * 128],
                E_all[:, tt * BL : (tt + 1) * BL],
                identity[:],
            )

        # ---- softmax (row-wise over free dim)
        negmax = const.tile([BL, 1], f32)
        nc.vector.tensor_reduce(
            negmax[:], erow[:], axis=mybir.AxisListType.X, op=Alu.max, negate=True
        )
        xrow = const.tile([BL, T], f32)
        nc.scalar.activation(xrow[:], erow[:], Act.Exp, bias=negmax[:], scale=1.0)
        wrow = const.tile([BL, T], f32)
        zsum = const.tile([BL, 1], f32)
        nc.vector.tensor_tensor(out=wrow[:], in0=xrow[:], in1=mask_f[:], op=Alu.mult)
        nc.vector.tensor_reduce(
            zsum[:], wrow[:], axis=mybir.AxisListType.X, op=Alu.add
        )
        rz = const.tile([BL, 1], f32)
        nc.vector.reciprocal(rz[:], zsum[:])
        attn = const.tile([BL, T], f32)
        nc.vector.tensor_scalar_mul(attn[:], wrow[:], rz[:])
        nc.sync.dma_start(attn_d.ap(), attn[:])

        # ---- transpose normalized attn to column layout:
        #      wcol[p, tt*BL+b] = attn[b, tt*128+p]  (already scaled by 1/Z)
        wcol_ps = psum_w.tile([128, NT * BL], f32)
        for tt in range(NT):
            nc.tensor.transpose(
                wcol_ps[:, tt * BL : (tt + 1) * BL],
                attn[:, tt * 128 : (tt + 1) * 128],
                identity[:BL, :BL],
            )
        wcol = const.tile([128, NT * BL], f32r)
        nc.scalar.copy(wcol[:], wcol_ps[:])

        # ---- context: ctx[b, h] = sum_t attn[b, t] * V[t, b, h] (float32r matmuls)
        for b in range(BL):
            vt = vpool.tile([128, NT, H], f32r, tag="vt")
            nc.sync.dma_start(vt[:], v_r[b])
            cps = psum_c.tile([1, H], f32)
            for tt in range(NT):
                nc.tensor.matmul(
                    cps[:],
                    wcol[:, tt * BL + b : tt * BL + b + 1],
                    vt[:, tt, :],
                    start=(tt == 0),
                    stop=(tt == NT - 1),
                )
            crow = prodp.tile([1, H], f32, tag="crow")
            nc.scalar.copy(crow[:], cps[:])
            nc.sync.dma_start(ctx_d.ap()[b : b + 1, :], crow[:])

    nc.compile()
    return nc


def _get_nc():
    if "nc" not in _cache:
        _cache["nc"] = _build_nc(debug=False)
    return _cache["nc"]


def _shard_inputs(query, key, value, mask):
    in_maps = []
    for i in range(NCORES):
        s = slice(i * BL, (i + 1) * BL)
        in_maps.append(
            {
                "query": np.ascontiguousarray(query[s]),
                "key": np.ascontiguousarray(key[:, s]),
                "value": np.ascontiguousarray(value[:, s]),
                "mask": np.ascontiguousarray(mask[s]),
            }
        )
    return in_maps


def run_sharded(query, key, value, mask, trace=False, **kw):
    from concourse.bass_utils import run_bass_kernel_spmd

    nc = _get_nc()
    in_maps = _shard_inputs(query, key, value, mask)
    res = run_bass_kernel_spmd(
        nc, in_maps, core_ids=list(range(NCORES)), trace=trace, **kw
    )
    context = np.concatenate([res.results[i]["out_ctx"] for i in range(NCORES)], axis=0)
    attn = np.concatenate([res.results[i]["out_attn"] for i in range(NCORES)], axis=0)
    return (context, attn), res


def kernel(query, key, value, mask):
    query = np.asarray(query, dtype=np.float32)
    key = np.asarray(key, dtype=np.float32)
    value = np.asarray(value, dtype=np.float32)
    mask = np.asarray(mask, dtype=np.int32)
    (context, attn), _ = run_sharded(query, key, value, mask, trace=False)
    return (context, attn)
